# revision 1
# baseline (speedup 1.0000x reference)
"""Cross-attention kernel for Trainium2 (Bass/Tile), data-parallel over batch.

Full inputs in, full outputs out. B=8 batch elements -> one per NeuronCore.
Per core: query [1024,1024], context [2048,768] -> out [1024,1024].

Layout strategy (per core):
  - queryT/contextT: feature-major via PE transposes.
  - qT[inner, nq], kT[inner, nkv] computed directly in feature-major layout.
  - v computed in natural [nkv, inner] layout with a ones-column interleaved
    per head, so the PV matmul's M=65 stationary also produces softmax
    row-sums as output row 64.
  - scores S^T[kv, q] per head with K=64 matmuls; two heads packed in the
    128-row PE array via tile_position row groups -> full array utilization.
  - exp on ScalarE (scores bounded ~8, no max subtraction needed).
  - normalization: PE broadcast of sums row (K=1 matmul with ones) + DVE
    reciprocal + multiply.
  - out = O @ w_out + b_out in natural layout, DMA'd out contiguously.
All matmuls in float32r (full PE rate, ~1e-4 matmul rel err); E/v in bf16.
"""

import numpy as np

import concourse.bass as bass
import concourse.tile as tile
from concourse import bacc, mybir
from concourse.alu_op_type import AluOpType
from concourse.bass_utils import run_bass_kernel_spmd
from concourse.masks import make_identity

NQ, QD, CD, NKV = 1024, 1024, 768, 2048
H, DH, INNER = 16, 64, 1024
SCALE = DH**-0.5
NQT, QDT, CDT, KVT, IT = NQ // 128, QD // 128, CD // 128, NKV // 128, INNER // 128
B = 8

f32 = mybir.dt.float32
f32r = mybir.dt.float32r
bf16 = mybir.dt.bfloat16
FT = mybir.ActivationFunctionType


def declare(nc):
    return dict(
        query=nc.dram_tensor("query", [NQ, QD], f32, kind="ExternalInput"),
        context=nc.dram_tensor("context", [NKV, CD], f32, kind="ExternalInput"),
        w_q=nc.dram_tensor("w_q", [QD, INNER], f32, kind="ExternalInput"),
        w_kv=nc.dram_tensor("w_kv", [CD, 2 * INNER], f32, kind="ExternalInput"),
        w_out=nc.dram_tensor("w_out", [INNER, QD], f32, kind="ExternalInput"),
        b_out=nc.dram_tensor("b_out", [QD], f32, kind="ExternalInput"),
        out=nc.dram_tensor("out", [NQ, QD], f32, kind="ExternalOutput"),
    )


def emit(nc, tc, T, upto="E"):
    query, context, w_q, w_kv = T["query"], T["context"], T["w_q"], T["w_kv"]
    w_out, b_out, out = T["w_out"], T["b_out"], T["out"]

    # Pools with interleaved (non-LIFO) lifetimes are managed manually.
    const = tc.alloc_tile_pool(name="const", bufs=1)
    OTp = tc.alloc_tile_pool(name="OTp", bufs=1)
    OT = [OTp.tile([128, NQ], f32r, name=f"OT{t}", tag=f"OT{t}") for t in range(IT)]
    qTp = tc.alloc_tile_pool(name="qTp", bufs=1)

    ident_f = const.tile([128, 128], f32, name="ident_f", tag="ident_f")
    make_identity(nc, ident_f)
    ident = const.tile([128, 128], f32r, name="ident", tag="ident")
    nc.vector.tensor_copy(ident, ident_f)
    ones64_f = const.tile([128, 64], f32, name="ones64_f", tag="ones64_f")
    nc.vector.memset(ones64_f, 1.0)
    ones64 = const.tile([128, 64], f32r, name="ones64", tag="ones64")
    nc.vector.tensor_copy(ones64, ones64_f)
    bias_bc = const.tile([128, QD], f32, name="bias", tag="bias")
    nc.sync.dma_start(bias_bc, b_out[:].partition_broadcast(128))

    qT = [qTp.tile([128, NQ], f32r, name=f"qT{m}", tag=f"qT{m}") for m in range(IT)]

    ctxTp = tc.alloc_tile_pool(name="ctxTp", bufs=1)
    ctxT = [
        ctxTp.tile([128, NKV], f32r, name=f"ctxT{j}", tag=f"ctxT{j}")
        for j in range(CDT)
    ]

    # ---- Phase A: load & transpose query and context ----
    with tc.tile_pool(name="qryTp", bufs=1) as qryTp:
        queryT = [
            qryTp.tile([128, NQ], f32r, name=f"qryT{j}", tag=f"qryT{j}")
            for j in range(QDT)
        ]
        with (
            tc.tile_pool(name="phA", bufs=2) as phA,
            tc.tile_pool(name="psA", bufs=8, space="PSUM") as psA,
        ):
            for i2 in range(NQT // 2):
                qnat = phA.tile([128, 2, QD], f32r, name="qnat", tag="qnat")
                q_src = bass.AP(
                    tensor=query,
                    offset=i2 * 256 * QD,
                    ap=[[QD, 128], [128 * QD, 2], [1, QD]],
                ).bitcast(f32r)
                nc.sync.dma_start(qnat, q_src)
                for r in range(2):
                    i = i2 * 2 + r
                    for j in range(QDT):
                        pt = psA.tile([128, 128], f32r, name="pt", tag="pt")
                        nc.tensor.transpose(
                            pt, qnat[:, r, j * 128 : (j + 1) * 128], ident
                        )
                        # split psum->SBUF copies across DVE and ACT (both idle-ish)
                        eng = nc.vector.tensor_copy if (i + j) % 2 else nc.scalar.copy
                        eng(queryT[j][:, i * 128 : (i + 1) * 128], pt)
            for i2 in range(KVT // 2):
                cnat = phA.tile([128, 2, CD], f32r, name="cnat", tag="cnat")
                c_src = bass.AP(
                    tensor=context,
                    offset=i2 * 256 * CD,
                    ap=[[CD, 128], [128 * CD, 2], [1, CD]],
                ).bitcast(f32r)
                nc.sync.dma_start(cnat, c_src)
                for r in range(2):
                    i = i2 * 2 + r
                    for j in range(CDT):
                        pt = psA.tile([128, 128], f32r, name="pt", tag="pt")
                        nc.tensor.transpose(
                            pt, cnat[:, r, j * 128 : (j + 1) * 128], ident
                        )
                        eng = nc.vector.tensor_copy if (i + j) % 2 else nc.scalar.copy
                        eng(ctxT[j][:, i * 128 : (i + 1) * 128], pt)

        # ---- Phase B: qT[m] = sum_j w_q[j-block, m-block].T @ queryT[j] ----
        # w_q loaded as per-m [128, QDT, 128] slices (one strided DMA each)
        with (
            tc.tile_pool(name="wq", bufs=2) as wqp,
            tc.tile_pool(name="psB", bufs=4, space="PSUM") as psB,
        ):
            for m in range(IT if upto >= "B" else 0):
                wqm = wqp.tile([128, QDT, 128], f32r, name="wqm", tag="wqm")
                wq_src = bass.AP(
                    tensor=w_q,
                    offset=m * 128,
                    ap=[[INNER, 128], [128 * INNER, QDT], [1, 128]],
                ).bitcast(f32r)
                nc.sync.dma_start(wqm, wq_src)
                for h in range(2):
                    ps = psB.tile([128, 512], f32, name="ps_b", tag="ps_b")
                    for j in range(QDT):
                        nc.tensor.matmul(
                            ps,
                            wqm[:, j, :],
                            queryT[j][:, h * 512 : (h + 1) * 512],
                            start=(j == 0),
                            stop=(j == QDT - 1),
                        )
                    nc.vector.tensor_copy(qT[m][:, h * 512 : (h + 1) * 512], ps)

    # ---- Phase C0: v natural (interleaved ones col), bf16 ----
    vp = tc.alloc_tile_pool(name="vp", bufs=1)
    v_sb = [
        vp.tile([128, H * 65], bf16, name=f"v{t}", tag=f"v{t}") for t in range(KVT)
    ]
    with (
        tc.tile_pool(name="wvp", bufs=1) as wvp,
        tc.tile_pool(name="psC", bufs=4, space="PSUM") as psC,
    ):
        wv = [
            wvp.tile([128, INNER], f32r, name=f"wv{j}", tag=f"wv{j}")
            for j in range(CDT)
        ]
        for j in range(CDT):
            nc.sync.dma_start(
                wv[j],
                w_kv[j * 128 : (j + 1) * 128, INNER : 2 * INNER].bitcast(f32r),
            )
        for t in range(KVT if upto >= "C" else 0):
            vt = v_sb[t].rearrange("p (h c) -> p h c", c=65)
            for h in range(2):
                ps = psC.tile([128, 512], f32, name="ps_c", tag="ps_c")
                for j in range(CDT):
                    nc.tensor.matmul(
                        ps,
                        ctxT[j][:, t * 128 : (t + 1) * 128],
                        wv[j][:, h * 512 : (h + 1) * 512],
                        start=(j == 0),
                        stop=(j == CDT - 1),
                    )
                nc.vector.tensor_copy(
                    vt[:, h * 8 : (h + 1) * 8, 0:64],
                    ps.rearrange("p (h c) -> p h c", c=64),
                )
            nc.vector.memset(vt[:, :, 64:65], 1.0)

    # ---- Phase D: attention per head pair ----
    with (
        tc.tile_pool(name="kTp", bufs=2) as kTp,
        tc.tile_pool(name="wks", bufs=2) as wkp,
        tc.tile_pool(name="Ep", bufs=8) as Ep,
        tc.tile_pool(name="norm", bufs=2) as normp,
        tc.tile_pool(name="psS", bufs=3, space="PSUM") as psS,
        tc.tile_pool(name="psK", bufs=1, space="PSUM") as psK,
        tc.tile_pool(name="psO", bufs=4, space="PSUM") as psO,
    ):
        for t in range(IT if upto >= "D" else 0):
            h0, h1 = 2 * t, 2 * t + 1
            # kT_t[128, NKV]: k^T for heads h0 (rows 0:64), h1 (rows 64:128)
            kT_t = kTp.tile([128, NKV], f32r, name="kT", tag="kT")
            # one strided DMA for all CDT row-blocks of this pair's w_k slice:
            # wk_t[p, j, c] = w_kv[j*128+p, t*128+c]
            wk_t = wkp.tile([128, CDT, 128], f32r, name="wk", tag="wk")
            wk_src = bass.AP(
                tensor=w_kv,
                offset=t * 128,
                ap=[[2 * INNER, 128], [128 * 2 * INNER, CDT], [1, 128]],
            ).bitcast(f32r)
            nc.sync.dma_start(wk_t, wk_src)
            wks = [wk_t[:, j, :] for j in range(CDT)]
            for n in range(4):
                pkt = psK.tile([128, 512], f32, name="ps_k", tag="ps_k")
                for j in range(CDT):
                    nc.tensor.matmul(
                        pkt,
                        wks[j],
                        ctxT[j][:, n * 512 : (n + 1) * 512],
                        start=(j == 0),
                        stop=(j == CDT - 1),
                    )
                nc.vector.tensor_copy(kT_t[:, n * 512 : (n + 1) * 512], pkt)
            pO = {}
            for hi in range(2):
                for half in range(2):
                    pO[(hi, half)] = psO.tile(
                        [65, 512], f32, name="ps_o", tag="ps_o"
                    )
            for kv in range(KVT):
                kvlo = kv * 128
                E0 = Ep.tile([128, NQ], bf16, name="E", tag="E")
                E1 = Ep.tile([128, NQ], bf16, name="E", tag="E")
                # alternate row groups (h0/h1) so each matmul's weight load
                # overlaps the previous matmul in the other row group
                for half in range(2):
                    lo = half * 512
                    for hi, E in ((0, E0), (1, E1)):
                        plo, phi = (0, 64) if hi == 0 else (64, 128)
                        tp = (0, 0) if hi == 0 else (64, 0)
                        pS = psS.tile([128, 512], f32, name="ps_s", tag="ps_s")
                        nc.tensor.matmul(
                            pS,
                            kT_t[plo:phi, kvlo : kvlo + 128],
                            qT[t][plo:phi, lo : lo + 512],
                            start=True,
                            stop=True,
                            tile_position=tp,
                        )
                        nc.scalar.activation(
                            E[:, lo : lo + 512], pS, FT.Exp, scale=SCALE
                        )
                for hi, E in ((0, E0), (1, E1)):
                    hh = h0 if hi == 0 else h1
                    for half in range(2):
                        lo = half * 512
                        nc.tensor.matmul(
                            pO[(hi, half)],
                            v_sb[kv][:, hh * 65 : hh * 65 + 65],
                            E[:, lo : lo + 512],
                            start=(kv == 0),
                            stop=(kv == KVT - 1),
                        )

            # normalize: rows 0:64 divided by sums row 64.
            # Copy psum -> SBUF first (frees PSUM banks for the next pair),
            # then broadcast the sums row via a K=1 matmul into freed psum
            # slots, reciprocal + multiply on DVE.
            for hi in range(2):
                oc = normp.tile([65, NQ], f32r, name="oc", tag="oc")
                for half in range(2):
                    lo = half * 512
                    nc.vector.tensor_copy(oc[:, lo : lo + 512], pO[(hi, half)])
                prb = {}
                for half in range(2):
                    lo = half * 512
                    prb[half] = psO.tile([64, 512], f32, name="ps_o", tag="ps_o")
                    nc.tensor.matmul(
                        prb[half],
                        ones64[64:65, :],
                        oc[64:65, lo : lo + 512],
                        start=True,
                        stop=True,
                    )
                rb = normp.tile([64, NQ], f32, name="rb", tag="rb", bufs=1)
                for half in range(2):
                    lo = half * 512
                    nc.vector.reciprocal(rb[:, lo : lo + 512], prb[half])
                if hi == 0:
                    dst = OT[t][0:64, :]
                else:
                    dst = normp.tile([64, NQ], f32r, name="otmp", tag="otmp", bufs=1)
                for half in range(2):
                    lo = half * 512
                    nc.vector.tensor_tensor(
                        dst[:, lo : lo + 512],
                        oc[0:64, lo : lo + 512],
                        rb[:, lo : lo + 512],
                        op=AluOpType.mult,
                    )
                if hi == 1:
                    nc.gpsimd.dma_start(OT[t][64:128, :], dst)

    vp.release()
    ctxTp.release()
    qTp.release()

    # ---- Phase E: out = O @ w_out + b_out ----
    with (
        tc.tile_pool(name="wo", bufs=1) as wop,
        tc.tile_pool(name="osb", bufs=3) as osbp,
        tc.tile_pool(name="psE", bufs=4, space="PSUM") as psE,
    ):
        wo = [
            wop.tile([128, QD], f32r, name=f"wo{i}", tag=f"wo{i}") for i in range(IT)
        ]
        for i in range(IT):
            nc.sync.dma_start(wo[i], w_out[i * 128 : (i + 1) * 128, :].bitcast(f32r))
        for m in range(NQT if upto >= "E" else 0):
            o_sb = osbp.tile([128, QD], f32, name="osb", tag="osb")
            for half in range(2):
                lo = half * 512
                ps = psE.tile([128, 512], f32, name="ps_e", tag="ps_e")
                for i in range(IT):
                    nc.tensor.matmul(
                        ps,
                        OT[i][:, m * 128 : (m + 1) * 128],
                        wo[i][:, lo : lo + 512],
                        start=(i == 0),
                        stop=(i == IT - 1),
                    )
                nc.vector.tensor_tensor(
                    o_sb[:, lo : lo + 512],
                    ps,
                    bias_bc[:, lo : lo + 512],
                    op=AluOpType.add,
                )
            nc.gpsimd.dma_start(out[m * 128 : (m + 1) * 128, :], o_sb)

    OTp.release()
    const.release()


def build(reps=1, upto="E"):
    nc = bacc.Bacc("TRN2", target_bir_lowering=False, debug=False)
    T = declare(nc)
    with tile.TileContext(nc) as tc:
        for _ in range(reps):
            emit(nc, tc, T, upto=upto)
    nc.compile()
    return nc


_nc_cache = None


def _get_nc():
    global _nc_cache
    if _nc_cache is None:
        _nc_cache = build()
    return _nc_cache


def kernel(query, context, w_q, w_kv, w_out, b_out, **run_kwargs):
    nc = _get_nc()
    query = np.ascontiguousarray(query, dtype=np.float32)
    context = np.ascontiguousarray(context, dtype=np.float32)
    shared = {
        "w_q": np.ascontiguousarray(w_q, dtype=np.float32),
        "w_kv": np.ascontiguousarray(w_kv, dtype=np.float32),
        "w_out": np.ascontiguousarray(w_out, dtype=np.float32),
        "b_out": np.ascontiguousarray(b_out, dtype=np.float32),
    }
    in_maps = [
        {"query": query[b], "context": context[b], **shared} for b in range(B)
    ]
    res = run_bass_kernel_spmd(nc, in_maps, core_ids=list(range(B)), **run_kwargs)
    out = np.stack([res.results[b]["out"] for b in range(B)])
    if run_kwargs:
        kernel.last_result = res
    return out



# revision 2
# speedup vs baseline: 1.1413x; 1.1413x over previous
"""Cross-attention kernel for Trainium2 (Bass/Tile), data-parallel over batch.

Full inputs in, full outputs out. B=8 batch elements -> one per NeuronCore.
Per core: query [1024,1024], context [2048,768] -> out [1024,1024].

Layout strategy (per core):
  - queryT/contextT: feature-major via PE transposes.
  - qT[inner, nq], kT[inner, nkv] computed directly in feature-major layout.
  - v computed in natural [nkv, inner] layout with a ones-column interleaved
    per head, so the PV matmul's M=65 stationary also produces softmax
    row-sums as output row 64.
  - scores S^T[kv, q] per head with K=64 matmuls; two heads packed in the
    128-row PE array via tile_position row groups -> full array utilization.
  - exp on ScalarE (scores bounded ~8, no max subtraction needed).
  - normalization: PE broadcast of sums row (K=1 matmul with ones) + DVE
    reciprocal + multiply.
  - out = O @ w_out + b_out in natural layout, DMA'd out contiguously.
All matmuls in float32r (full PE rate, ~1e-4 matmul rel err); E/v in bf16.
"""

import numpy as np

import concourse.bass as bass
import concourse.tile as tile
from concourse import bacc, mybir
from concourse.alu_op_type import AluOpType
from concourse.bass_utils import run_bass_kernel_spmd
from concourse.masks import make_identity

NQ, QD, CD, NKV = 1024, 1024, 768, 2048
H, DH, INNER = 16, 64, 1024
SCALE = DH**-0.5
NQT, QDT, CDT, KVT, IT = NQ // 128, QD // 128, CD // 128, NKV // 128, INNER // 128
B = 8

f32 = mybir.dt.float32
f32r = mybir.dt.float32r
bf16 = mybir.dt.bfloat16
FT = mybir.ActivationFunctionType


def declare(nc):
    return dict(
        query=nc.dram_tensor("query", [NQ, QD], f32, kind="ExternalInput"),
        context=nc.dram_tensor("context", [NKV, CD], f32, kind="ExternalInput"),
        w_q=nc.dram_tensor("w_q", [QD, INNER], f32, kind="ExternalInput"),
        w_kv=nc.dram_tensor("w_kv", [CD, 2 * INNER], f32, kind="ExternalInput"),
        w_out=nc.dram_tensor("w_out", [INNER, QD], f32, kind="ExternalInput"),
        b_out=nc.dram_tensor("b_out", [QD], f32, kind="ExternalInput"),
        out=nc.dram_tensor("out", [NQ, QD], f32, kind="ExternalOutput"),
    )


def emit(nc, tc, T, upto="E"):
    query, context, w_q, w_kv = T["query"], T["context"], T["w_q"], T["w_kv"]
    w_out, b_out, out = T["w_out"], T["b_out"], T["out"]

    # Pools with interleaved (non-LIFO) lifetimes are managed manually.
    const = tc.alloc_tile_pool(name="const", bufs=1)
    OTp = tc.alloc_tile_pool(name="OTp", bufs=1)
    OT = [OTp.tile([128, NQ], f32r, name=f"OT{t}", tag=f"OT{t}") for t in range(IT)]
    qTp = tc.alloc_tile_pool(name="qTp", bufs=1)

    ident_f = const.tile([128, 128], f32, name="ident_f", tag="ident_f")
    make_identity(nc, ident_f)
    ident = const.tile([128, 128], f32r, name="ident", tag="ident")
    nc.vector.tensor_copy(ident, ident_f)
    ones64_f = const.tile([128, 64], f32, name="ones64_f", tag="ones64_f")
    nc.vector.memset(ones64_f, 1.0)
    ones64 = const.tile([128, 64], f32r, name="ones64", tag="ones64")
    nc.vector.tensor_copy(ones64, ones64_f)
    bias_bc = const.tile([128, QD], f32, name="bias", tag="bias")
    nc.sync.dma_start(bias_bc, b_out[:].partition_broadcast(128))

    qT = [qTp.tile([128, NQ], f32r, name=f"qT{m}", tag=f"qT{m}") for m in range(IT)]

    ctxTp = tc.alloc_tile_pool(name="ctxTp", bufs=1)
    ctxT = [
        ctxTp.tile([128, NKV], f32r, name=f"ctxT{j}", tag=f"ctxT{j}")
        for j in range(CDT)
    ]

    # ---- Phase A: load & transpose query and context ----
    with tc.tile_pool(name="qryTp", bufs=1) as qryTp:
        queryT = [
            qryTp.tile([128, NQ], f32r, name=f"qryT{j}", tag=f"qryT{j}")
            for j in range(QDT)
        ]
        with (
            tc.tile_pool(name="phA", bufs=2) as phA,
            tc.tile_pool(name="psA", bufs=8, space="PSUM") as psA,
        ):
            for i2 in range(NQT // 2):
                qnat = phA.tile([128, 2, QD], f32r, name="qnat", tag="qnat")
                q_src = bass.AP(
                    tensor=query,
                    offset=i2 * 256 * QD,
                    ap=[[QD, 128], [128 * QD, 2], [1, QD]],
                ).bitcast(f32r)
                nc.sync.dma_start(qnat, q_src)
                for r in range(2):
                    i = i2 * 2 + r
                    for j in range(QDT):
                        pt = psA.tile([128, 128], f32r, name="pt", tag="pt")
                        nc.tensor.transpose(
                            pt, qnat[:, r, j * 128 : (j + 1) * 128], ident
                        )
                        # split psum->SBUF copies across DVE and ACT (both idle-ish)
                        eng = nc.vector.tensor_copy if (i + j) % 2 else nc.scalar.copy
                        eng(queryT[j][:, i * 128 : (i + 1) * 128], pt)
            for i2 in range(KVT // 2):
                cnat = phA.tile([128, 2, CD], f32r, name="cnat", tag="cnat")
                c_src = bass.AP(
                    tensor=context,
                    offset=i2 * 256 * CD,
                    ap=[[CD, 128], [128 * CD, 2], [1, CD]],
                ).bitcast(f32r)
                nc.sync.dma_start(cnat, c_src)
                for r in range(2):
                    i = i2 * 2 + r
                    for j in range(CDT):
                        pt = psA.tile([128, 128], f32r, name="pt", tag="pt")
                        nc.tensor.transpose(
                            pt, cnat[:, r, j * 128 : (j + 1) * 128], ident
                        )
                        eng = nc.vector.tensor_copy if (i + j) % 2 else nc.scalar.copy
                        eng(ctxT[j][:, i * 128 : (i + 1) * 128], pt)

        # ---- Phase B: qT[m] = sum_j w_q[j-block, m-block].T @ queryT[j] ----
        # w_q loaded as per-m [128, QDT, 128] slices (one strided DMA each)
        with (
            tc.tile_pool(name="wq", bufs=2) as wqp,
            tc.tile_pool(name="psB", bufs=4, space="PSUM") as psB,
        ):
            for m in range(IT if upto >= "B" else 0):
                wqm = wqp.tile([128, QDT, 128], f32r, name="wqm", tag="wqm")
                wq_src = bass.AP(
                    tensor=w_q,
                    offset=m * 128,
                    ap=[[INNER, 128], [128 * INNER, QDT], [1, 128]],
                ).bitcast(f32r)
                nc.sync.dma_start(wqm, wq_src)
                for h in range(2):
                    ps = psB.tile([128, 512], f32, name="ps_b", tag="ps_b")
                    for j in range(QDT):
                        nc.tensor.matmul(
                            ps,
                            wqm[:, j, :],
                            queryT[j][:, h * 512 : (h + 1) * 512],
                            start=(j == 0),
                            stop=(j == QDT - 1),
                        )
                    nc.vector.tensor_copy(qT[m][:, h * 512 : (h + 1) * 512], ps)

    # ---- Phase C0: v natural (interleaved ones col), bf16 ----
    vp = tc.alloc_tile_pool(name="vp", bufs=1)
    v_sb = [
        vp.tile([128, H * 65], bf16, name=f"v{t}", tag=f"v{t}") for t in range(KVT)
    ]
    with (
        tc.tile_pool(name="wvp", bufs=1) as wvp,
        tc.tile_pool(name="psC", bufs=4, space="PSUM") as psC,
    ):
        wv = [
            wvp.tile([128, INNER], f32r, name=f"wv{j}", tag=f"wv{j}")
            for j in range(CDT)
        ]
        for j in range(CDT):
            nc.sync.dma_start(
                wv[j],
                w_kv[j * 128 : (j + 1) * 128, INNER : 2 * INNER].bitcast(f32r),
            )
        for t in range(KVT if upto >= "C" else 0):
            vt = v_sb[t].rearrange("p (h c) -> p h c", c=65)
            for h in range(2):
                ps = psC.tile([128, 512], f32, name="ps_c", tag="ps_c")
                for j in range(CDT):
                    nc.tensor.matmul(
                        ps,
                        ctxT[j][:, t * 128 : (t + 1) * 128],
                        wv[j][:, h * 512 : (h + 1) * 512],
                        start=(j == 0),
                        stop=(j == CDT - 1),
                    )
                nc.vector.tensor_copy(
                    vt[:, h * 8 : (h + 1) * 8, 0:64],
                    ps.rearrange("p (h c) -> p h c", c=64),
                )
            nc.vector.memset(vt[:, :, 64:65], 1.0)

    # ---- Phase D: attention per head pair ----
    with (
        tc.tile_pool(name="kTp", bufs=2) as kTp,
        tc.tile_pool(name="wks", bufs=2) as wkp,
        tc.tile_pool(name="Ep", bufs=8) as Ep,
        tc.tile_pool(name="norm", bufs=2) as normp,
        tc.tile_pool(name="psS", bufs=2, space="PSUM") as psS,
        tc.tile_pool(name="psO", bufs=4, space="PSUM") as psO,
    ):
        for t in range(IT if upto >= "D" else 0):
            h0, h1 = 2 * t, 2 * t + 1
            # kT_t[128, NKV]: k^T for heads h0 (rows 0:64), h1 (rows 64:128)
            kT_t = kTp.tile([128, NKV], f32r, name="kT", tag="kT")
            # one strided DMA for all CDT row-blocks of this pair's w_k slice:
            # wk_t[p, j, c] = w_kv[j*128+p, t*128+c]
            wk_t = wkp.tile([128, CDT, 128], f32r, name="wk", tag="wk")
            wk_src = bass.AP(
                tensor=w_kv,
                offset=t * 128,
                ap=[[2 * INNER, 128], [128 * 2 * INNER, CDT], [1, 128]],
            ).bitcast(f32r)
            nc.sync.dma_start(wk_t, wk_src)
            wks = [wk_t[:, j, :] for j in range(CDT)]
            # kT psum comes from the scores pool (2-bank tiles hold 2 n-blocks)
            for nn in range(2):
                pkt = psS.tile([128, 1024], f32, name="ps_s", tag="ps_s")
                for n2 in range(2):
                    n = nn * 2 + n2
                    for j in range(CDT):
                        nc.tensor.matmul(
                            pkt[:, n2 * 512 : (n2 + 1) * 512],
                            wks[j],
                            ctxT[j][:, n * 512 : (n + 1) * 512],
                            start=(j == 0),
                            stop=(j == CDT - 1),
                        )
                nc.vector.tensor_copy(
                    kT_t[:, nn * 1024 : (nn + 1) * 1024], pkt
                )
            pO = {}
            for hi in range(2):
                for half in range(2):
                    pO[(hi, half)] = psO.tile(
                        [65, 512], f32, name="ps_o", tag="ps_o"
                    )
            for kv in range(KVT):
                kvlo = kv * 128
                E0 = Ep.tile([128, NQ], bf16, name="E", tag="E")
                E1 = Ep.tile([128, NQ], bf16, name="E", tag="E")
                # each head's scores land in one 2-bank psum tile so a single
                # N=1024 exp covers them (halves ACT instruction count);
                # alternate row groups (h0/h1) so the matmuls pair up
                # concurrently on the PE (tile_position row groups)
                pS = {
                    0: psS.tile([128, 1024], f32, name="ps_s", tag="ps_s"),
                    1: psS.tile([128, 1024], f32, name="ps_s", tag="ps_s"),
                }
                for half in range(2):
                    lo = half * 512
                    for hi in (0, 1):
                        plo, phi = (0, 64) if hi == 0 else (64, 128)
                        tp = (0, 0) if hi == 0 else (64, 0)
                        nc.tensor.matmul(
                            pS[hi][:, lo : lo + 512],
                            kT_t[plo:phi, kvlo : kvlo + 128],
                            qT[t][plo:phi, lo : lo + 512],
                            start=True,
                            stop=True,
                            tile_position=tp,
                        )
                for hi, E in ((0, E0), (1, E1)):
                    nc.scalar.activation(E, pS[hi], FT.Exp, scale=SCALE)
                for hi, E in ((0, E0), (1, E1)):
                    hh = h0 if hi == 0 else h1
                    for half in range(2):
                        lo = half * 512
                        nc.tensor.matmul(
                            pO[(hi, half)],
                            v_sb[kv][:, hh * 65 : hh * 65 + 65],
                            E[:, lo : lo + 512],
                            start=(kv == 0),
                            stop=(kv == KVT - 1),
                        )

            # normalize: rows 0:64 divided by sums row 64.
            # Copy psum -> SBUF first (frees PSUM banks for the next pair),
            # then broadcast the sums row via a K=1 matmul into freed psum
            # slots, reciprocal + multiply on DVE.
            for hi in range(2):
                oc = normp.tile([65, NQ], f32r, name="oc", tag="oc")
                for half in range(2):
                    lo = half * 512
                    nc.vector.tensor_copy(oc[:, lo : lo + 512], pO[(hi, half)])
                prb = {}
                for half in range(2):
                    lo = half * 512
                    prb[half] = psO.tile([64, 512], f32, name="ps_o", tag="ps_o")
                    nc.tensor.matmul(
                        prb[half],
                        ones64[64:65, :],
                        oc[64:65, lo : lo + 512],
                        start=True,
                        stop=True,
                    )
                rb = normp.tile([64, NQ], f32, name="rb", tag="rb", bufs=1)
                for half in range(2):
                    lo = half * 512
                    nc.vector.reciprocal(rb[:, lo : lo + 512], prb[half])
                if hi == 0:
                    dst = OT[t][0:64, :]
                else:
                    dst = normp.tile([64, NQ], f32r, name="otmp", tag="otmp", bufs=1)
                for half in range(2):
                    lo = half * 512
                    nc.vector.tensor_tensor(
                        dst[:, lo : lo + 512],
                        oc[0:64, lo : lo + 512],
                        rb[:, lo : lo + 512],
                        op=AluOpType.mult,
                    )
                if hi == 1:
                    nc.gpsimd.dma_start(OT[t][64:128, :], dst)

    vp.release()
    ctxTp.release()
    qTp.release()

    # ---- Phase E: out = O @ w_out + b_out ----
    with (
        tc.tile_pool(name="wo", bufs=1) as wop,
        tc.tile_pool(name="osb", bufs=3) as osbp,
        tc.tile_pool(name="psE", bufs=4, space="PSUM") as psE,
    ):
        wo = [
            wop.tile([128, QD], f32r, name=f"wo{i}", tag=f"wo{i}") for i in range(IT)
        ]
        for i in range(IT):
            nc.sync.dma_start(wo[i], w_out[i * 128 : (i + 1) * 128, :].bitcast(f32r))
        for m in range(NQT if upto >= "E" else 0):
            o_sb = osbp.tile([128, QD], f32, name="osb", tag="osb")
            for half in range(2):
                lo = half * 512
                ps = psE.tile([128, 512], f32, name="ps_e", tag="ps_e")
                for i in range(IT):
                    nc.tensor.matmul(
                        ps,
                        OT[i][:, m * 128 : (m + 1) * 128],
                        wo[i][:, lo : lo + 512],
                        start=(i == 0),
                        stop=(i == IT - 1),
                    )
                nc.vector.tensor_tensor(
                    o_sb[:, lo : lo + 512],
                    ps,
                    bias_bc[:, lo : lo + 512],
                    op=AluOpType.add,
                )
            nc.gpsimd.dma_start(out[m * 128 : (m + 1) * 128, :], o_sb)

    OTp.release()
    const.release()


def build(reps=1, upto="E"):
    nc = bacc.Bacc("TRN2", target_bir_lowering=False, debug=False)
    T = declare(nc)
    with tile.TileContext(nc) as tc:
        for _ in range(reps):
            emit(nc, tc, T, upto=upto)
    nc.compile()
    return nc


_nc_cache = None


def _get_nc():
    global _nc_cache
    if _nc_cache is None:
        _nc_cache = build()
    return _nc_cache


def kernel(query, context, w_q, w_kv, w_out, b_out, **run_kwargs):
    nc = _get_nc()
    query = np.ascontiguousarray(query, dtype=np.float32)
    context = np.ascontiguousarray(context, dtype=np.float32)
    shared = {
        "w_q": np.ascontiguousarray(w_q, dtype=np.float32),
        "w_kv": np.ascontiguousarray(w_kv, dtype=np.float32),
        "w_out": np.ascontiguousarray(w_out, dtype=np.float32),
        "b_out": np.ascontiguousarray(b_out, dtype=np.float32),
    }
    in_maps = [
        {"query": query[b], "context": context[b], **shared} for b in range(B)
    ]
    res = run_bass_kernel_spmd(nc, in_maps, core_ids=list(range(B)), **run_kwargs)
    out = np.stack([res.results[b]["out"] for b in range(B)])
    if run_kwargs:
        kernel.last_result = res
    return out



# revision 3
# speedup vs baseline: 1.9579x; 1.7155x over previous
"""Cross-attention kernel for Trainium2 (Bass/Tile), data-parallel over batch.

Full inputs in, full outputs out. B=8 batch elements -> one per NeuronCore.
Per core: query [1024,1024], context [2048,768] -> out [1024,1024].

Layout strategy (per core):
  - queryT/contextT: feature-major via PE transposes.
  - qT[inner, nq], kT[inner, nkv] computed directly in feature-major layout.
  - v computed in natural [nkv, inner] layout with a ones-column interleaved
    per head, so the PV matmul's M=65 stationary also produces softmax
    row-sums as output row 64.
  - scores S^T[kv, q] per head pair; the two heads' matmuls go to distinct
    PE row groups (tile_position) so they run concurrently on HW.
  - each head's scores for a kv block land in one 2-bank PSUM tile so a
    single N=1024 exp on ScalarE covers them (ACT instr count is the
    phase-D bottleneck).
  - kT for pair t+1 is computed inside pair t's kv loop so the scalar
    engine never idles between pairs.
  - DMAs are spread across engine queues (sync/scalar/gpsimd) so input
    loads and weight prefetches run in parallel.
  - normalization: PE broadcast of sums row (K=1 matmul with ones) + DVE
    reciprocal + multiply.
  - out = O @ w_out + b_out in natural layout, DMA'd out contiguously.
All matmuls in float32r (full PE rate, ~1e-4 matmul rel err); E/v in bf16.
"""

import numpy as np

import concourse.bass as bass
import concourse.tile as tile
from concourse import bacc, mybir
from concourse.alu_op_type import AluOpType
from concourse.bass_utils import run_bass_kernel_spmd
from concourse.masks import make_identity

NQ, QD, CD, NKV = 1024, 1024, 768, 2048
H, DH, INNER = 16, 64, 1024
SCALE = DH**-0.5
NQT, QDT, CDT, KVT, IT = NQ // 128, QD // 128, CD // 128, NKV // 128, INNER // 128
B = 8

f32 = mybir.dt.float32
f32r = mybir.dt.float32r
bf16 = mybir.dt.bfloat16
FT = mybir.ActivationFunctionType


def declare(nc):
    return dict(
        query=nc.dram_tensor("query", [NQ, QD], f32, kind="ExternalInput"),
        context=nc.dram_tensor("context", [NKV, CD], f32, kind="ExternalInput"),
        w_q=nc.dram_tensor("w_q", [QD, INNER], f32, kind="ExternalInput"),
        w_kv=nc.dram_tensor("w_kv", [CD, 2 * INNER], f32, kind="ExternalInput"),
        w_out=nc.dram_tensor("w_out", [INNER, QD], f32, kind="ExternalInput"),
        b_out=nc.dram_tensor("b_out", [QD], f32, kind="ExternalInput"),
        out=nc.dram_tensor("out", [NQ, QD], f32, kind="ExternalOutput"),
    )


def emit(nc, tc, T, upto="E"):
    query, context, w_q, w_kv = T["query"], T["context"], T["w_q"], T["w_kv"]
    w_out, b_out, out = T["w_out"], T["b_out"], T["out"]

    # Pools with interleaved (non-LIFO) lifetimes are managed manually.
    const = tc.alloc_tile_pool(name="const", bufs=1)
    OTp = tc.alloc_tile_pool(name="OTp", bufs=1)
    OT = [OTp.tile([128, NQ], f32r, name=f"OT{t}", tag=f"OT{t}") for t in range(IT)]
    qTp = tc.alloc_tile_pool(name="qTp", bufs=1)

    ident_f = const.tile([128, 128], f32, name="ident_f", tag="ident_f")
    make_identity(nc, ident_f)
    ident = const.tile([128, 128], f32r, name="ident", tag="ident")
    nc.vector.tensor_copy(ident, ident_f)
    ones64_f = const.tile([128, 64], f32, name="ones64_f", tag="ones64_f")
    nc.vector.memset(ones64_f, 1.0)
    ones64 = const.tile([128, 64], f32r, name="ones64", tag="ones64")
    nc.vector.tensor_copy(ones64, ones64_f)
    bias_bc = const.tile([128, QD], f32, name="bias", tag="bias")
    nc.sync.dma_start(bias_bc, b_out[:].partition_broadcast(128))

    qT = [qTp.tile([128, NQ], f32r, name=f"qT{m}", tag=f"qT{m}") for m in range(IT)]

    ctxTp = tc.alloc_tile_pool(name="ctxTp", bufs=1)
    ctxT = [
        ctxTp.tile([128, NKV], f32r, name=f"ctxT{j}", tag=f"ctxT{j}")
        for j in range(CDT)
    ]

    # context chunks are DMA'd on the gpsimd queue so they stream in
    # concurrently with the query transposes + phase B below.
    phAc = tc.alloc_tile_pool(name="phAc", bufs=3)
    cnats = []

    def emit_cnat_dma(i2):
        cnat = phAc.tile([128, 2, CD], f32r, name="cnat", tag="cnat")
        c_src = bass.AP(
            tensor=context,
            offset=i2 * 256 * CD,
            ap=[[CD, 128], [128 * CD, 2], [1, CD]],
        ).bitcast(f32r)
        nc.gpsimd.dma_start(cnat, c_src)
        cnats.append(cnat)

    for i2 in range(2):
        emit_cnat_dma(i2)

    # ---- Phase A-q: load & transpose query ----
    with tc.tile_pool(name="qryTp", bufs=1) as qryTp:
        queryT = [
            qryTp.tile([128, NQ], f32r, name=f"qryT{j}", tag=f"qryT{j}")
            for j in range(QDT)
        ]
        with (
            tc.tile_pool(name="phA", bufs=2) as phA,
            tc.tile_pool(name="psA", bufs=8, space="PSUM") as psA,
        ):
            for i2 in range(NQT // 2):
                qnat = phA.tile([128, 2, QD], f32r, name="qnat", tag="qnat")
                q_src = bass.AP(
                    tensor=query,
                    offset=i2 * 256 * QD,
                    ap=[[QD, 128], [128 * QD, 2], [1, QD]],
                ).bitcast(f32r)
                nc.sync.dma_start(qnat, q_src)
                for r in range(2):
                    i = i2 * 2 + r
                    for j in range(QDT):
                        pt = psA.tile([128, 128], f32r, name="pt", tag="pt")
                        nc.tensor.transpose(
                            pt, qnat[:, r, j * 128 : (j + 1) * 128], ident
                        )
                        # split psum->SBUF copies across DVE and ACT (both idle)
                        eng = nc.vector.tensor_copy if (i + j) % 2 else nc.scalar.copy
                        eng(queryT[j][:, i * 128 : (i + 1) * 128], pt)

        # ---- Phase B: qT[m] = sum_j w_q[j-block, m-block].T @ queryT[j] ----
        # w_q loaded as per-m [128, QDT, 128] slices (one strided DMA each,
        # on the scalar queue so they overlap the query loads)
        with (
            tc.tile_pool(name="wq", bufs=2) as wqp,
            tc.tile_pool(name="psB", bufs=4, space="PSUM") as psB,
        ):
            for m in range(IT if upto >= "B" else 0):
                wqm = wqp.tile([128, QDT, 128], f32r, name="wqm", tag="wqm")
                wq_src = bass.AP(
                    tensor=w_q,
                    offset=m * 128,
                    ap=[[INNER, 128], [128 * INNER, QDT], [1, 128]],
                ).bitcast(f32r)
                nc.scalar.dma_start(wqm, wq_src)
                for h in range(2):
                    ps = psB.tile([128, 512], f32, name="ps_b", tag="ps_b")
                    for j in range(QDT):
                        nc.tensor.matmul(
                            ps,
                            wqm[:, j, :],
                            queryT[j][:, h * 512 : (h + 1) * 512],
                            start=(j == 0),
                            stop=(j == QDT - 1),
                        )
                    nc.vector.tensor_copy(qT[m][:, h * 512 : (h + 1) * 512], ps)

    # ---- Phase A-c: transpose context (chunks already streaming in) ----
    with tc.tile_pool(name="psAc", bufs=8, space="PSUM") as psAc:
        for i2 in range(KVT // 2):
            if i2 + 2 < KVT // 2:
                emit_cnat_dma(i2 + 2)
            cnat = cnats[i2]
            for r in range(2):
                i = i2 * 2 + r
                for j in range(CDT):
                    pt = psAc.tile([128, 128], f32r, name="pt", tag="pt")
                    nc.tensor.transpose(
                        pt, cnat[:, r, j * 128 : (j + 1) * 128], ident
                    )
                    eng = nc.vector.tensor_copy if (i + j) % 2 else nc.scalar.copy
                    eng(ctxT[j][:, i * 128 : (i + 1) * 128], pt)
    phAc.release()

    # ---- Phase C: v natural (interleaved ones col), bf16 ----
    vp = tc.alloc_tile_pool(name="vp", bufs=1)
    v_sb = [
        vp.tile([128, H * 65], bf16, name=f"v{t}", tag=f"v{t}") for t in range(KVT)
    ]
    with (
        tc.tile_pool(name="wvp", bufs=1) as wvp,
        tc.tile_pool(name="psC", bufs=4, space="PSUM") as psC,
    ):
        wv = [
            wvp.tile([128, INNER], f32r, name=f"wv{j}", tag=f"wv{j}")
            for j in range(CDT)
        ]
        for j in range(CDT):
            nc.scalar.dma_start(
                wv[j],
                w_kv[j * 128 : (j + 1) * 128, INNER : 2 * INNER].bitcast(f32r),
            )
        for t in range(KVT if upto >= "C" else 0):
            vt = v_sb[t].rearrange("p (h c) -> p h c", c=65)
            for h in range(2):
                ps = psC.tile([128, 512], f32, name="ps_c", tag="ps_c")
                for j in range(CDT):
                    nc.tensor.matmul(
                        ps,
                        ctxT[j][:, t * 128 : (t + 1) * 128],
                        wv[j][:, h * 512 : (h + 1) * 512],
                        start=(j == 0),
                        stop=(j == CDT - 1),
                    )
                nc.vector.tensor_copy(
                    vt[:, h * 8 : (h + 1) * 8, 0:64],
                    ps.rearrange("p (h c) -> p h c", c=64),
                )
            nc.vector.memset(vt[:, :, 64:65], 1.0)

    # ---- Phase D: attention per head pair; kT pipelined one pair ahead ----
    with (
        tc.tile_pool(name="kTp", bufs=2) as kTp,
        tc.tile_pool(name="wks", bufs=2) as wkp,
        tc.tile_pool(name="Ep", bufs=8) as Ep,
        tc.tile_pool(name="norm", bufs=2) as normp,
        tc.tile_pool(name="psS", bufs=2, space="PSUM") as psS,
        tc.tile_pool(name="psO", bufs=4, space="PSUM") as psO,
    ):
        kT_sb = {}
        wk_sb = {}

        def emit_wk_dma(t):
            # wk_t[p, j, c] = w_kv[j*128+p, t*128+c], one strided DMA
            wk_t = wkp.tile([128, CDT, 128], f32r, name="wk", tag="wk")
            wk_src = bass.AP(
                tensor=w_kv,
                offset=t * 128,
                ap=[[2 * INNER, 128], [128 * 2 * INNER, CDT], [1, 128]],
            ).bitcast(f32r)
            nc.gpsimd.dma_start(wk_t, wk_src)
            wk_sb[t] = wk_t
            kT_sb[t] = kTp.tile([128, NKV], f32r, name="kT", tag="kT")

        def emit_kT_half(t, nn):
            # one 2-bank psum tile accumulates 2 n-blocks, then one DVE copy
            pkt = psS.tile([128, 1024], f32, name="ps_s", tag="ps_s")
            for n2 in range(2):
                n = nn * 2 + n2
                for j in range(CDT):
                    nc.tensor.matmul(
                        pkt[:, n2 * 512 : (n2 + 1) * 512],
                        wk_sb[t][:, j, :],
                        ctxT[j][:, n * 512 : (n + 1) * 512],
                        start=(j == 0),
                        stop=(j == CDT - 1),
                    )
            nc.vector.tensor_copy(kT_sb[t][:, nn * 1024 : (nn + 1) * 1024], pkt)

        npairs = IT if upto >= "D" else 0
        if npairs:
            emit_wk_dma(0)
            emit_kT_half(0, 0)
            emit_kT_half(0, 1)
        for t in range(npairs):
            h0, h1 = 2 * t, 2 * t + 1
            kT_t = kT_sb[t]
            pO = {}
            for hi in range(2):
                for half in range(2):
                    pO[(hi, half)] = psO.tile(
                        [65, 512], f32, name="ps_o", tag="ps_o"
                    )
            for kv in range(KVT):
                if t + 1 < npairs:
                    if kv == 2:
                        emit_wk_dma(t + 1)
                    elif kv == 6:
                        emit_kT_half(t + 1, 0)
                    elif kv == 11:
                        emit_kT_half(t + 1, 1)
                kvlo = kv * 128
                E0 = Ep.tile([128, NQ], bf16, name="E", tag="E")
                E1 = Ep.tile([128, NQ], bf16, name="E", tag="E")
                # each head's scores land in one 2-bank psum tile so a single
                # N=1024 exp covers them; alternate row groups (h0/h1) so the
                # matmul pairs run concurrently on the PE
                pS = {
                    0: psS.tile([128, 1024], f32, name="ps_s", tag="ps_s"),
                    1: psS.tile([128, 1024], f32, name="ps_s", tag="ps_s"),
                }
                for half in range(2):
                    lo = half * 512
                    for hi in (0, 1):
                        plo, phi = (0, 64) if hi == 0 else (64, 128)
                        tp = (0, 0) if hi == 0 else (64, 0)
                        nc.tensor.matmul(
                            pS[hi][:, lo : lo + 512],
                            kT_t[plo:phi, kvlo : kvlo + 128],
                            qT[t][plo:phi, lo : lo + 512],
                            start=True,
                            stop=True,
                            tile_position=tp,
                        )
                for hi, E in ((0, E0), (1, E1)):
                    nc.scalar.activation(E, pS[hi], FT.Exp, scale=SCALE)
                for hi, E in ((0, E0), (1, E1)):
                    hh = h0 if hi == 0 else h1
                    for half in range(2):
                        lo = half * 512
                        nc.tensor.matmul(
                            pO[(hi, half)],
                            v_sb[kv][:, hh * 65 : hh * 65 + 65],
                            E[:, lo : lo + 512],
                            start=(kv == 0),
                            stop=(kv == KVT - 1),
                        )

            # normalize: rows 0:64 divided by sums row 64.
            # Copy psum -> SBUF first (frees PSUM banks for the next pair),
            # then broadcast the sums row via a K=1 matmul into freed psum
            # slots, reciprocal + multiply on DVE.
            for hi in range(2):
                oc = normp.tile([65, NQ], f32r, name="oc", tag="oc")
                for half in range(2):
                    lo = half * 512
                    nc.vector.tensor_copy(oc[:, lo : lo + 512], pO[(hi, half)])
                prb = {}
                for half in range(2):
                    lo = half * 512
                    prb[half] = psO.tile([64, 512], f32, name="ps_o", tag="ps_o")
                    nc.tensor.matmul(
                        prb[half],
                        ones64[64:65, :],
                        oc[64:65, lo : lo + 512],
                        start=True,
                        stop=True,
                    )
                rb = normp.tile([64, NQ], f32, name="rb", tag="rb", bufs=1)
                for half in range(2):
                    lo = half * 512
                    nc.vector.reciprocal(rb[:, lo : lo + 512], prb[half])
                if hi == 0:
                    dst = OT[t][0:64, :]
                else:
                    dst = normp.tile([64, NQ], f32r, name="otmp", tag="otmp", bufs=1)
                for half in range(2):
                    lo = half * 512
                    nc.vector.tensor_tensor(
                        dst[:, lo : lo + 512],
                        oc[0:64, lo : lo + 512],
                        rb[:, lo : lo + 512],
                        op=AluOpType.mult,
                    )
                if hi == 1:
                    nc.gpsimd.dma_start(OT[t][64:128, :], dst)

    vp.release()
    ctxTp.release()
    qTp.release()

    # ---- Phase E: out = O @ w_out + b_out ----
    with (
        tc.tile_pool(name="wo", bufs=1) as wop,
        tc.tile_pool(name="osb", bufs=3) as osbp,
        tc.tile_pool(name="psE", bufs=4, space="PSUM") as psE,
    ):
        wo = [
            wop.tile([128, QD], f32r, name=f"wo{i}", tag=f"wo{i}") for i in range(IT)
        ]
        for i in range(IT):
            nc.sync.dma_start(wo[i], w_out[i * 128 : (i + 1) * 128, :].bitcast(f32r))
        for m in range(NQT if upto >= "E" else 0):
            o_sb = osbp.tile([128, QD], f32, name="osb", tag="osb")
            for half in range(2):
                lo = half * 512
                ps = psE.tile([128, 512], f32, name="ps_e", tag="ps_e")
                for i in range(IT):
                    nc.tensor.matmul(
                        ps,
                        OT[i][:, m * 128 : (m + 1) * 128],
                        wo[i][:, lo : lo + 512],
                        start=(i == 0),
                        stop=(i == IT - 1),
                    )
                nc.vector.tensor_tensor(
                    o_sb[:, lo : lo + 512],
                    ps,
                    bias_bc[:, lo : lo + 512],
                    op=AluOpType.add,
                )
            nc.gpsimd.dma_start(out[m * 128 : (m + 1) * 128, :], o_sb)

    OTp.release()
    const.release()


def build(reps=1, upto="E"):
    nc = bacc.Bacc("TRN2", target_bir_lowering=False, debug=False)
    T = declare(nc)
    with tile.TileContext(nc) as tc:
        for _ in range(reps):
            emit(nc, tc, T, upto=upto)
    nc.compile()
    return nc


_nc_cache = None


def _get_nc():
    global _nc_cache
    if _nc_cache is None:
        _nc_cache = build()
    return _nc_cache


def kernel(query, context, w_q, w_kv, w_out, b_out, **run_kwargs):
    nc = _get_nc()
    query = np.ascontiguousarray(query, dtype=np.float32)
    context = np.ascontiguousarray(context, dtype=np.float32)
    shared = {
        "w_q": np.ascontiguousarray(w_q, dtype=np.float32),
        "w_kv": np.ascontiguousarray(w_kv, dtype=np.float32),
        "w_out": np.ascontiguousarray(w_out, dtype=np.float32),
        "b_out": np.ascontiguousarray(b_out, dtype=np.float32),
    }
    in_maps = [
        {"query": query[b], "context": context[b], **shared} for b in range(B)
    ]
    res = run_bass_kernel_spmd(nc, in_maps, core_ids=list(range(B)), **run_kwargs)
    out = np.stack([res.results[b]["out"] for b in range(B)])
    if run_kwargs:
        kernel.last_result = res
    return out
